# revision 1
# baseline (speedup 1.0000x reference)
"""Causal single-head attention (B=4, S=4096, E=32, H=64) on 8 TRN2 NeuronCores.

Sharding: core c handles batch b=c//2 and query parity p=c%2 (query chunks of
256 rows, chunks p, p+2, ..., p+14 of that batch). Causal work per chunk grows
linearly, so parity interleaving balances the load while keeping control flow
identical on every core (one SPMD NEFF); all per-core differences are input
data (host-permuted queries and host-built masks).

Device algorithm per core (slot s = 0..7, 256 queries each, E_s = 4s+4 key
chunks of 128):
  QT/KT = [W;b].T @ [x^T;1]  (bias folded into the matmul via ones row)
  V     = [x^T;1].T @ [Wv,0;bv,1]  (ones column appended -> denominator)
  ST[j,q] = KT_chunk.T @ QT_slot   (scores transposed, contraction = H)
  PT = exp(ST/8)  (ACT, scale folded into activation; no max subtraction --
                   |scores/8| < ~2 for this distribution)
  PT *= mask      (only last 4 chunks of each slot are not fully causal)
  ACC[h,q] += V_chunk.T @ PT       (PSUM accumulation over chunks)
  out[q,h] = transpose(ACC)[:, :64] / transpose(ACC)[:, 64]
"""

import numpy as np
import ml_dtypes

B, S, E, H = 4, 4096, 32, 64
P = 128
NQ = S // 2          # queries per core
SLOTS = 8            # 256-query slots per core
SQ = 256             # queries per slot

_BF16 = ml_dtypes.bfloat16

_cache = {}


def _mark(nc, label):
    """Record (label, #instructions) build marks for sim-profiling."""
    marks = getattr(nc, "_phase_marks", None)
    if marks is None:
        marks = []
        nc._phase_marks = marks
    marks.append((label, len(nc.inst_map)))


def _build_bass(
    reps=1,
    scb=4,            # key chunks per score-psum batch (one ACT call each)
    score_bufs=3,
    pt_bufs=4,
    mask_engine="vector",   # vector | gpsimd
    proj_split=False,       # route first proj psum->sbuf copies to ACT
    slot_order=None,
    skip_exp=False,         # timing probe: omit exp+mask+PV+fin
    skip_qk=False,          # timing probe: omit QK too (with skip_exp)
    skip_pv=False,          # timing probe: omit PV+fin only
    skip_mask=False,        # timing probe: omit mask multiplies
    qk_pack=True,           # row-pack QK pairs into 64-row array halves
    host_div=True,          # ship [h,q] accumulators; host divides+transposes
):
    skip_pv = skip_pv or skip_exp
    skip_mask = skip_mask or skip_exp
    """Build the SPMD NEFF. reps>1 repeats the whole computation (with a
    scheduling barrier between reps) -- used only for wall-clock timing."""
    import concourse.tile as tile
    from concourse import bacc, mybir
    from concourse.masks import make_identity

    f32 = mybir.dt.float32
    bf16 = mybir.dt.bfloat16
    Exp = mybir.ActivationFunctionType.Exp
    SCALE = 1.0 / float(np.sqrt(H))

    nc = bacc.Bacc(None, target_bir_lowering=False)
    xqT = nc.dram_tensor("xqT", [E + 1, NQ], bf16, kind="ExternalInput")
    xkvT = nc.dram_tensor("xkvT", [E + 1, S], bf16, kind="ExternalInput")
    wqkv = nc.dram_tensor("wqkv", [E + 1, 2 * H + H + 1], bf16,
                          kind="ExternalInput")
    mask4 = nc.dram_tensor("mask4", [P, 4, SQ], bf16, kind="ExternalInput")
    if host_div:
        out = nc.dram_tensor("out", [SLOTS, H + 1, SQ], f32,
                             kind="ExternalOutput")
    else:
        out = nc.dram_tensor("out", [NQ, H], f32, kind="ExternalOutput")

    NKC = S // P  # 32 key chunks
    if slot_order is None:
        slot_order = list(range(SLOTS))

    with tile.TileContext(nc) as tc:
        with (
            tc.tile_pool(name="const", bufs=1) as cpool,
            tc.tile_pool(name="pt", bufs=pt_bufs) as ptpool,
            tc.tile_pool(name="fin", bufs=2) as finpool,
            tc.tile_pool(name="ps_score", bufs=score_bufs, space="PSUM") as spool,
            tc.tile_pool(name="ps_pv", bufs=2 if host_div else 1,
                         space="PSUM") as pvpool,
            tc.tile_pool(name="ps_tr", bufs=1, space="PSUM") as trpool,
        ):
            for rep in range(reps):
                if rep:
                    tc.strict_bb_all_engine_barrier()
                # ---- load inputs. Each DMA costs ~625ns of serialized HWDGE
                #      queue time regardless of size -> few, big DMAs, in
                #      dependency-criticality order.
                w_sb = cpool.tile([E + 1, 2 * H + H + 1], bf16, tag="w")
                nc.sync.dma_start(w_sb[:], wqkv[:])
                xq_sb = cpool.tile([E + 1, NQ], bf16, tag="xq")
                nc.sync.dma_start(xq_sb[:], xqT[:])
                wq_sb = w_sb[:, 0:H]
                wk_sb = w_sb[:, H:2 * H]
                wv_sb = w_sb[:, 2 * H:2 * H + H + 1]
                xkv_sb = cpool.tile([E + 1, S], bf16, tag="xkv")
                nc.sync.dma_start(xkv_sb[:], xkvT[:])
                mask_sb = cpool.tile([P, 4, SQ], bf16, tag="mask")
                nc.sync.dma_start(mask_sb[:], mask4[:])
                xq_t = [xq_sb[:, c * 512:(c + 1) * 512]
                        for c in range(NQ // 512)]
                xkv_t = [xkv_sb[:, c * 512:(c + 1) * 512]
                         for c in range(S // 512)]
                if not host_div:
                    ident = cpool.tile([P, P], f32, tag="ident")
                    make_identity(nc, ident[:])

                _mark(nc, "load")

                def proj_copy(idx, dst, src):
                    if proj_split and idx < 3:
                        nc.scalar.copy(dst, src)
                    else:
                        nc.vector.tensor_copy(dst, src)

                # ---- projections are emitted interleaved with the slot
                #      loop (PE runs its queue in order; emitting all
                #      projections first would stall attention ~10us).
                QKP = P if qk_pack else H
                qt_t = [cpool.tile([QKP, 512], bf16, tag=f"qt{c}", name=f"qt{c}")
                        for c in range(NQ // 512)]
                kt_t = [cpool.tile([QKP, 512], bf16, tag=f"kt{c}", name=f"kt{c}")
                        for c in range(S // 512)]
                v_t = [cpool.tile([P, 4, H + 1], bf16, tag=f"v{g}", name=f"v{g}")
                       for g in range(NKC // 4)]

                def proj_qk_mm(ps, w, x):
                    # With qk_pack, produce [w.T@x; w.T@x] stacked on
                    # partitions via two col-packed concurrent matmuls.
                    nc.tensor.matmul(
                        ps[:H, :], w, x, start=True, stop=True,
                    )
                    if qk_pack:
                        nc.tensor.matmul(
                            ps[H:2 * H, :], w, x, start=True, stop=True,
                            tile_position=(0, H),
                        )

                def emit_proj(c):
                    if c < NQ // 512:
                        ps = spool.tile([QKP, 512], f32, tag="score",
                                        name=f"ps_q{c}")
                        proj_qk_mm(ps, wq_sb, xq_t[c])
                        proj_copy(c, qt_t[c][:], ps[:])
                    ps = spool.tile([QKP, 512], f32, tag="score", name=f"ps_k{c}")
                    proj_qk_mm(ps, wk_sb, xkv_t[c])
                    proj_copy(c + 1, kt_t[c][:], ps[:])
                    ps = spool.tile([P, 4, H + 1], f32, tag="score",
                                    name=f"ps_v{c}")
                    for i in range(4):
                        nc.tensor.matmul(
                            ps[:, i, :], xkv_t[c][:, i * P:(i + 1) * P], wv_sb,
                            start=True, stop=True,
                        )
                    proj_copy(c, v_t[c][:], ps[:])

                _mark(nc, "proj")
                # ---- main attention loop (proj chunk s emitted just
                #      before slot s; slot s depends on chunks 0..s) ----
                emitted_proj = set()

                def ensure_proj(upto):
                    for c in range(upto + 1):
                        if c not in emitted_proj:
                            emitted_proj.add(c)
                            emit_proj(c)

                def emit_fin(s, acc_ps):
                    acc_sb = finpool.tile([H + 1, SQ], f32, tag="acc_sb",
                                          name=f"acc_sb{s}")
                    nc.vector.tensor_copy(acc_sb[:], acc_ps[:])
                    if host_div:
                        nc.sync.dma_start(out[s, :, :], acc_sb[:])
                        _mark(nc, f"slot{s}_fin")
                        return
                    for hh in range(2):
                        tr_ps = trpool.tile([P, H + 1], f32, tag="tr",
                                            name=f"tr{s}_{hh}")
                        nc.tensor.transpose(
                            tr_ps[:], acc_sb[:, hh * P:(hh + 1) * P],
                            ident[: H + 1, : H + 1],
                        )
                        rec = finpool.tile([P, 1], f32, tag="rec",
                                           name=f"rec{s}_{hh}")
                        nc.vector.reciprocal(rec[:], tr_ps[:, H:H + 1])
                        o_sb = finpool.tile([P, H], f32, tag="o",
                                            name=f"o{s}_{hh}")
                        nc.vector.tensor_scalar_mul(o_sb[:], tr_ps[:, :H],
                                                    rec[:])
                        r0 = s * SQ + hh * P
                        nc.sync.dma_start(out[r0:r0 + P, :], o_sb[:])
                    _mark(nc, f"slot{s}_fin")

                def emit_pv(s, b0, nb, ext, pt_sb, acc_ps):
                    if skip_pv:
                        return
                    for i in range(nb):
                        jc = b0 + i
                        nc.tensor.matmul(
                            acc_ps[:], v_t[jc // 4][:, jc % 4, :],
                            pt_sb[:, i, :],
                            start=(jc == 0), stop=(jc == ext - 1),
                            skip_group_check=True,
                        )
                    if b0 + nb == ext:
                        _mark(nc, f"slot{s}_main")
                        emit_fin(s, acc_ps)

                # Flat software pipeline over all (slot, batch) items with a
                # one-batch emission lookahead: PE's FIFO sees QK(k+1) before
                # PV(k), so ACT's exp stream never stalls at slot boundaries.
                batches = []
                for s in slot_order:
                    ext = 4 * s + 4
                    for b0 in range(0, ext, scb):
                        batches.append((s, b0, min(scb, ext - b0), ext))

                acc_of = {}
                pending = None  # (s, b0, nb, ext, pt_sb, acc_ps)
                for (s, b0, nb, ext) in batches:
                    if b0 == 0:
                        ensure_proj(s)
                        if not skip_pv:
                            acc_of[s] = pvpool.tile(
                                [H + 1, SQ], f32, tag="acc", name=f"acc{s}")
                        else:
                            acc_of[s] = None
                    qs = qt_t[s // 2][:, (s % 2) * SQ:(s % 2 + 1) * SQ]
                    st_ps = spool.tile([P, scb, SQ], f32, tag="score",
                                       name=f"st{s}_{b0}")
                    if qk_pack and not skip_qk and nb == 4:
                        # pairs (0,2) and (1,3): the packed partners write
                        # different PSUM banks and use different array halves
                        for a in (0, 1):
                            for half, i in ((0, a), (1, a + 2)):
                                jc = b0 + i
                                kts = kt_t[jc // 4][
                                    half * H:(half + 1) * H,
                                    (jc % 4) * P:(jc % 4 + 1) * P,
                                ]
                                nc.tensor.matmul(
                                    st_ps[:, i, :], kts,
                                    qs[half * H:(half + 1) * H, :],
                                    start=True, stop=True,
                                )
                    else:
                        for i in range(nb):
                            if skip_qk:
                                break
                            jc = b0 + i
                            nc.tensor.matmul(
                                st_ps[:, i, :],
                                kt_t[jc // 4][:H, (jc % 4) * P:(jc % 4 + 1) * P],
                                qs[:H, :],
                                start=True, stop=True,
                            )
                    if pending is not None:
                        emit_pv(*pending)
                    pt_sb = ptpool.tile([P, scb, SQ], bf16, tag="pt",
                                        name=f"pt{s}_{b0}")
                    if not skip_exp:
                        nc.scalar.activation(
                            pt_sb[:, :nb, :], st_ps[:, :nb, :], Exp, scale=SCALE,
                        )
                    for i in range(nb):
                        jc = b0 + i
                        k = jc - (ext - 4)
                        if k >= 0 and not skip_mask:
                            eng = (
                                nc.gpsimd if mask_engine == "gpsimd"
                                else nc.vector
                            )
                            eng.tensor_mul(
                                pt_sb[:, i, :], pt_sb[:, i, :],
                                mask_sb[:, k, :],
                            )
                    pending = (s, b0, nb, ext, pt_sb, acc_of[s])
                if pending is not None:
                    emit_pv(*pending)

    nc.compile()
    return nc


def _host_inputs(x, Wq, bq, Wk, bk, Wv, bv):
    """Build the 8 per-core input maps."""
    ones_q = np.ones((1, NQ), np.float32)
    ones_s = np.ones((1, S), np.float32)
    wq_in = np.concatenate([Wq, bq[None, :]], axis=0)
    wk_in = np.concatenate([Wk, bk[None, :]], axis=0)
    wv_full = np.zeros((E + 1, H + 1), np.float32)
    wv_full[:E, :H] = Wv
    wv_full[E, :H] = bv
    wv_full[E, H] = 1.0
    wqkv_in = np.concatenate([wq_in, wk_in, wv_full], axis=1).astype(_BF16)

    r = np.arange(P)[:, None]
    f = np.arange(SQ)[None, :]
    m0 = (r <= f).astype(np.float32)
    m1 = (r + P <= f).astype(np.float32)
    zz = np.zeros((P, SQ), np.float32)
    oo = np.ones((P, SQ), np.float32)
    masks = [
        np.stack([m0, m1, zz, zz]).astype(_BF16),  # parity 0
        np.stack([oo, oo, m0, m1]).astype(_BF16),  # parity 1
    ]

    in_maps = []
    for c in range(8):
        b, p = divmod(c, 2)
        xb = x[b]  # [S, E]
        rows = np.concatenate(
            [np.arange(u * SQ, (u + 1) * SQ) for u in range(p, 16, 2)]
        )
        xq = xb[rows]  # [NQ, E]
        xqT = np.concatenate([xq.T, ones_q], axis=0).astype(_BF16)
        xkvT = np.concatenate([xb.T, ones_s], axis=0).astype(_BF16)
        in_maps.append({
            "xqT": np.ascontiguousarray(xqT),
            "xkvT": np.ascontiguousarray(xkvT),
            "wqkv": wqkv_in,
            "mask4": masks[p].transpose(1, 0, 2).copy(),  # [P, 4, SQ]
        })
    return in_maps


def _unshard(results, host_div=True):
    out = np.empty((B, S, H), np.float32)
    for c in range(8):
        b, p = divmod(c, 2)
        oc = results[c]["out"]
        for si, u in enumerate(range(p, 16, 2)):
            if host_div:
                acc = oc[si]  # [H+1, SQ]
                out[b, u * SQ:(u + 1) * SQ, :] = (acc[:H] / acc[H:H + 1]).T
            else:
                out[b, u * SQ:(u + 1) * SQ, :] = oc[si * SQ:(si + 1) * SQ, :]
    return out


def _get_runner(nc):
    """Build (once) a jitted 8-core executor for nc; returns a function
    taking in_maps and returning per-core output dicts. Mirrors
    bass2jax.run_bass_via_pjrt but caches the jit across calls."""
    import jax
    from jax.sharding import Mesh, PartitionSpec
    from jax.experimental.shard_map import shard_map
    from concourse import mybir
    from concourse.bass2jax import (
        _bass_exec_p,
        install_neuronx_cc_hook,
        partition_id_tensor,
    )

    install_neuronx_cc_hook()
    n_cores = 8
    partition_name = (
        nc.partition_id_tensor.name if nc.partition_id_tensor else None
    )
    in_names, out_names, out_avals = [], [], []
    for alloc in nc.m.functions[0].allocations:
        if not isinstance(alloc, mybir.MemoryLocationSet):
            continue
        name = alloc.memorylocations[0].name
        if alloc.kind == "ExternalInput":
            if name != partition_name:
                in_names.append(name)
        elif alloc.kind == "ExternalOutput":
            out_names.append(name)
            out_avals.append(
                jax.core.ShapedArray(
                    tuple(alloc.tensor_shape), mybir.dt.np(alloc.dtype)
                )
            )
    n_params = len(in_names)
    all_names = list(in_names) + list(out_names)
    if partition_name is not None:
        all_names.append(partition_name)

    def _body(*args):
        operands = list(args)
        if partition_name is not None:
            operands.append(partition_id_tensor())
        outs = _bass_exec_p.bind(
            *operands,
            out_avals=tuple(out_avals),
            in_names=tuple(all_names),
            out_names=tuple(out_names),
            lowering_input_output_aliases=(),
            sim_require_finite=True,
            sim_require_nnan=True,
            nc=nc,
        )
        return tuple(outs)

    devices = jax.devices()[:n_cores]
    mesh = Mesh(np.asarray(devices), ("core",))
    nouts = len(out_names)
    sharded = jax.jit(
        shard_map(
            _body,
            mesh=mesh,
            in_specs=(PartitionSpec("core"),) * (n_params + nouts),
            out_specs=(PartitionSpec("core"),) * nouts,
            check_rep=False,
        ),
        keep_unused=True,
    )

    def run(in_maps):
        concat_in = [
            np.concatenate(
                [np.asarray(in_maps[c][name]) for c in range(n_cores)], axis=0
            )
            for name in in_names
        ]
        concat_zero = [
            np.zeros((n_cores * av.shape[0], *av.shape[1:]), av.dtype)
            for av in out_avals
        ]
        outs = sharded(*concat_in, *concat_zero)
        return [
            {
                name: np.asarray(outs[i]).reshape(
                    n_cores, *out_avals[i].shape
                )[c]
                for i, name in enumerate(out_names)
            }
            for c in range(n_cores)
        ]

    return run


def kernel(x, Wq, bq, Wk, bk, Wv, bv):
    x = np.asarray(x, np.float32)
    Wq = np.asarray(Wq, np.float32)
    bq = np.asarray(bq, np.float32)
    Wk = np.asarray(Wk, np.float32)
    bk = np.asarray(bk, np.float32)
    Wv = np.asarray(Wv, np.float32)
    bv = np.asarray(bv, np.float32)

    if "nc" not in _cache:
        _cache["nc"] = _build_bass()
    nc = _cache["nc"]

    in_maps = _host_inputs(x, Wq, bq, Wk, bk, Wv, bv)
    try:
        if "runner" not in _cache:
            _cache["runner"] = _get_runner(nc)
        results = _cache["runner"](in_maps)
    except Exception:
        # fall back to the stock execution path
        _cache.pop("runner", None)
        from concourse.bass_utils import run_bass_kernel_spmd

        results = run_bass_kernel_spmd(
            nc, in_maps, core_ids=list(range(8))
        ).results
    return _unshard(results)



# revision 17
# speedup vs baseline: 1.0523x; 1.0523x over previous
"""Causal single-head attention (B=4, S=4096, E=32, H=64) on 8 TRN2 NeuronCores.

Sharding: core c handles batch b=c//2 and query parity p=c%2 (query chunks of
256 rows, chunks p, p+2, ..., p+14 of that batch). Causal work per chunk grows
linearly, so parity interleaving balances the load while keeping control flow
identical on every core (one SPMD NEFF); all per-core differences are input
data (host-permuted queries and host-built masks).

v2 changes vs v1:
  - Q/K/V projections moved to the host (tiny 32x64 GEMMs); the device
    receives ready-made bf16 qt/kt (row-duplicated for PE array-half packing)
    and v(+ones) tiles. Frees ~7us of PE and ~8us of DVE per core.
  - exp is split across ACT (native, clean batches) and DVE (Schraudolph
    bit-trick: i16 = round(s*C1 + C2) reinterpreted as bf16 ~= exp(s/8),
    max rel err 3.3%, end-to-end ~1e-2). Diagonal batches go to DVE where
    the causal mask is FUSED into the same instruction:
    scalar_tensor_tensor out_i16 = (s*C1) + maskC2[k,q], maskC2 = C2 for
    allowed pairs and C2-1e6 for masked ones; the f32->i16 convert saturates
    to -32768 = bf16 -0.0, so masked probs are exactly -0.0.
  - No device mask multiplies, no projection copies; DVE only does exp and
    the final accumulator copies.

Device algorithm per core (slot s = 0..7, 256 queries each, ext = 4s+4 key
chunks of 128):
  ST[j,q] = KT_chunk.T @ QT_slot   (PE; packed pairs on array row halves)
  PT = exp(ST/8) via ACT (scale folded) or DVE (bit-trick, mask fused)
  ACC[h,q] += V_chunk.T @ PT       (PE, PSUM accumulation; 65th V col = ones
                                    -> denominator row)
  out[s] = ACC  (DVE copy + DMA; host divides + transposes)
"""

import numpy as np
import ml_dtypes

B, S, E, H = 4, 4096, 32, 64
P = 128
NQ = S // 2          # queries per core
SLOTS = 8            # 256-query slots per core
SQ = 256             # queries per slot
NKC = S // P         # 32 key chunks
SCB = 4              # key chunks per score batch

_BF16 = ml_dtypes.bfloat16

LOG2E = float(np.log2(np.e))
C1 = 128.0 * LOG2E / 8.0      # bf16-Schraudolph multiplier (incl. 1/sqrt(H))
SIGMA = 5.5                   # minimax centering of the linear-mantissa error
C2 = 16256.0 - SIGMA
MASK_NEG = -1.0e6             # drives the i16 convert into -32768 -> bf16 -0.0

_cache = {}


def _mark(nc, label):
    """Record (label, #instructions) build marks for sim-profiling."""
    marks = getattr(nc, "_phase_marks", None)
    if marks is None:
        marks = []
        nc._phase_marks = marks
    marks.append((label, len(nc.inst_map)))


def _assign_engines(n_dve_clean=8, n_pool_clean=0):
    """Per-batch exp engine. Diagonal (mask-carrying) batches -> DVE (fused
    mask). Of the 28 clean batches, n_dve_clean go to DVE and n_pool_clean
    to GPSIMD, interleaved greedily by projected engine finish time."""
    batches = []
    for s in range(SLOTS):
        ext = 4 * s + 4
        for b0 in range(0, ext, SCB):
            batches.append((s, b0, min(SCB, ext - b0), ext))
    # greedy: walk in emission order, keep running engine loads
    load = {"act": 0.0, "dve": 0.0, "pool": 0.0}
    cost = {"act": 1.147, "dve": 1.20, "pool": 1.55}
    n_clean = sum(1 for (s, b0, nb, ext) in batches if b0 + nb != ext)
    quota = {"act": n_clean - n_dve_clean - n_pool_clean,
             "dve": n_dve_clean, "pool": n_pool_clean}
    eng = {}
    for (s, b0, nb, ext) in batches:
        if b0 + nb == ext:
            eng[(s, b0)] = "dve"
            load["dve"] += 1.33
            continue
        avail = [e for e in ("act", "dve", "pool") if quota[e] > 0]
        pick = min(avail, key=lambda e: load[e] + cost[e])
        quota[pick] -= 1
        load[pick] += cost[pick]
        eng[(s, b0)] = pick
    return batches, eng


def _build_bass(
    reps=1,
    score_bufs=3,
    pt_bufs=4,
    n_dve_clean=8,
    n_pool_clean=0,
    psum_dma_out=False,    # DMA from PSUM is not supported by the DGE path
    fin_engine="vector",   # acc copy engine: vector | scalar | gpsimd
    skip_exp=False,        # timing probe: omit exp+PV+fin
    skip_qk=False,         # timing probe: omit QK too (with skip_exp)
    skip_pv=False,         # timing probe: omit PV+fin only
):
    skip_pv = skip_pv or skip_exp
    """Build the SPMD NEFF. reps>1 repeats the whole computation (with a
    scheduling barrier between reps) -- used only for wall-clock timing."""
    import concourse.tile as tile
    from concourse import bacc, mybir

    f32 = mybir.dt.float32
    bf16 = mybir.dt.bfloat16
    i16 = mybir.dt.int16
    Exp = mybir.ActivationFunctionType.Exp
    Mult = mybir.AluOpType.mult
    Add = mybir.AluOpType.add
    SCALE = 1.0 / float(np.sqrt(H))

    nc = bacc.Bacc(None, target_bir_lowering=False)
    # host-projected inputs (see _host_inputs for layouts).
    # qk interleaves query and key tiles in dependency-need order so each
    # DMA delivers a self-sufficient prefix of the work:
    #   [qt s0-1 | kt g0-1 | qt s2-3 | kt g2-3 | qt s4-7 | kt g4-7]
    # kt is NOT duplicated: packed QK pairs read rows 0-63 only for chunk
    # positions {0,1} within each 4-chunk group and rows 64-127 only for
    # {2,3}, so the two row-halves carry different chunks (half the bytes).
    qk = nc.dram_tensor("qk", [P, NQ + S // 2], bf16, kind="ExternalInput")
    vt = nc.dram_tensor("vt", [P, NKC // 4, 4, H + 1], bf16,
                        kind="ExternalInput")
    # tri = C2/C2-1e6 triangular masks for diag-batch chunks {0,1}; cvec =
    # the per-core constant (C2 for parity 1, C2-1e6 for parity 0) applied
    # to diag-batch chunks {2,3} (host permutes keys so this layout is
    # parity-independent).
    tri = nc.dram_tensor("tri", [P, 2, SQ], f32, kind="ExternalInput")
    cvec = nc.dram_tensor("cvec", [P, 1], f32, kind="ExternalInput")
    out = nc.dram_tensor("out", [SLOTS, H + 1, SQ], f32, kind="ExternalOutput")

    batches, eng_of = _assign_engines(n_dve_clean, n_pool_clean)

    with tile.TileContext(nc) as tc:
        with (
            tc.tile_pool(name="const", bufs=1) as cpool,
            tc.tile_pool(name="pt", bufs=pt_bufs) as ptpool,
            tc.tile_pool(name="fin", bufs=2) as finpool,
            tc.tile_pool(name="ps_score", bufs=score_bufs, space="PSUM") as spool,
            tc.tile_pool(name="ps_pv", bufs=2, space="PSUM") as pvpool,
        ):
            for rep in range(reps):
                if rep:
                    tc.strict_bb_all_engine_barrier()
                # dummy activation: forces the exp table load at t=0, under
                # the input DMAs, instead of stalling the first real exp
                dummy = cpool.tile([1, 2], f32, tag="dummy")
                nc.scalar.activation(dummy[:, 0:1], dummy[:, 1:2], Exp)

                # ---- input DMAs, split fine-grained in need order (HWDGE
                #      issues serialize at ~625ns each; transfers serialize
                #      on the DMA engines, so early pieces are kept small).
                qk_sb = cpool.tile([P, NQ + S // 2], bf16, tag="qk")
                v_sb = cpool.tile([P, NKC // 4, 4, H + 1], bf16, tag="v")
                tri_sb = cpool.tile([P, 2, SQ], f32, tag="tri")
                cv_sb = cpool.tile([P, 1], f32, tag="cv")
                nc.sync.dma_start(qk_sb[:, 0:1024], qk[:, 0:1024])    # s01 g01
                nc.sync.dma_start(tri_sb[:], tri[:])
                nc.sync.dma_start(cv_sb[:], cvec[:])
                nc.sync.dma_start(v_sb[:, 0:2], vt[:, 0:2])           # g0-1
                nc.sync.dma_start(qk_sb[:, 1024:2048], qk[:, 1024:2048])
                nc.sync.dma_start(v_sb[:, 2:4], vt[:, 2:4])           # g2-3
                nc.sync.dma_start(qk_sb[:, 2048:], qk[:, 2048:])      # s47 g47
                nc.sync.dma_start(v_sb[:, 4:], vt[:, 4:])             # g4-7

                def qt_col(s):
                    return (s * SQ if s < 2 else
                            1024 + (s - 2) * SQ if s < 4 else
                            2048 + (s - 4) * SQ)

                def kt_slice(jc):
                    g, pos = divmod(jc, 4)
                    r0 = (pos // 2) * H
                    c0 = (512 + g * 256 if g < 2 else
                          1536 + (g - 2) * 256 if g < 4 else
                          3072 + (g - 4) * 256) + (pos % 2) * P
                    return qk_sb[r0:r0 + H, c0:c0 + P]

                _mark(nc, "load")

                def emit_fin(s, acc_ps):
                    if psum_dma_out:
                        nc.sync.dma_start(out[s, :, :], acc_ps[:])
                    else:
                        acc_sb = finpool.tile([H + 1, SQ], f32, tag="acc_sb",
                                              name=f"acc_sb{s}")
                        if fin_engine == "scalar":
                            nc.scalar.copy(acc_sb[:], acc_ps[:])
                        elif fin_engine == "gpsimd":
                            nc.gpsimd.tensor_copy(acc_sb[:], acc_ps[:])
                        else:
                            nc.vector.tensor_copy(acc_sb[:], acc_ps[:])
                        nc.sync.dma_start(out[s, :, :], acc_sb[:])
                    _mark(nc, f"slot{s}_fin")

                def emit_pv(s, b0, nb, ext, pt_sb, acc_ps):
                    if skip_pv:
                        return
                    # diag batches: const chunks {2,3} are exp'd first (see
                    # below), triangles {0,1} last -> issue PV in that order
                    # so the tail chain is as short as possible.
                    order = (2, 3, 0, 1) if b0 + nb == ext else range(nb)
                    order = [i for i in order if i < nb]
                    for n, i in enumerate(order):
                        jc = b0 + i
                        nc.tensor.matmul(
                            acc_ps[:], v_sb[:, jc // 4, jc % 4, :],
                            pt_sb[:, i, :].bitcast(bf16),
                            start=(b0 == 0 and n == 0),
                            stop=(b0 + nb == ext and n == len(order) - 1),
                            skip_group_check=True,
                        )
                    if b0 + nb == ext:
                        _mark(nc, f"slot{s}_main")
                        emit_fin(s, acc_ps)

                # Flat software pipeline over all (slot, batch) items with a
                # one-batch emission lookahead: PE's FIFO sees QK(k+1) before
                # PV(k), so the exp streams never stall at slot boundaries.
                acc_of = {}
                pending = None  # (s, b0, nb, ext, pt_sb, acc_ps)
                for (s, b0, nb, ext) in batches:
                    if b0 == 0:
                        if not skip_pv:
                            acc_of[s] = pvpool.tile(
                                [H + 1, SQ], f32, tag="acc", name=f"acc{s}")
                        else:
                            acc_of[s] = None
                    qs = qk_sb[:, qt_col(s):qt_col(s) + SQ]
                    st_ps = spool.tile([P, SCB, SQ], f32, tag="score",
                                       name=f"st{s}_{b0}")
                    if not skip_qk and nb == 4:
                        # packed pairs (0,2) and (1,3): partners use the two
                        # 64-row array halves concurrently. kt chunk position
                        # i sits on row-half i//2 of kt_sb by construction.
                        for a in (0, 1):
                            for half, i in ((0, a), (1, a + 2)):
                                nc.tensor.matmul(
                                    st_ps[:, i, :], kt_slice(b0 + i),
                                    qs[half * H:(half + 1) * H, :],
                                    start=True, stop=True,
                                )
                    if pending is not None:
                        emit_pv(*pending)
                    pt_sb = ptpool.tile([P, SCB, SQ], i16, tag="pt",
                                        name=f"pt{s}_{b0}")
                    if not skip_exp:
                        if eng_of[(s, b0)] == "act":
                            nc.scalar.activation(
                                pt_sb[:, :nb, :].bitcast(bf16),
                                st_ps[:, :nb, :], Exp, scale=SCALE,
                            )
                        elif eng_of[(s, b0)] == "pool":
                            nc.gpsimd.tensor_scalar(
                                pt_sb[:, :nb, :], st_ps[:, :nb, :], C1, C2,
                                Mult, Add,
                            )
                        elif b0 + nb == ext:
                            # diagonal batch. Chunks {2,3}: constant bias
                            # (C2 on parity-1 cores, C2-1e6 = fully masked on
                            # parity-0); chunks {0,1}: triangular mask fused
                            # via the saturating f32->i16 convert. Consts
                            # first -- the tail-critical triangles run last.
                            nc.vector.tensor_scalar(
                                pt_sb[:, 2:nb, :], st_ps[:, 2:nb, :], C1,
                                cv_sb[:, 0:1], Mult, Add,
                            )
                            nc.vector.scalar_tensor_tensor(
                                pt_sb[:, 0:2, :], st_ps[:, 0:2, :], C1,
                                tri_sb[:], Mult, Add,
                            )
                        else:
                            nc.vector.tensor_scalar(
                                pt_sb[:, :nb, :], st_ps[:, :nb, :], C1, C2,
                                Mult, Add,
                            )
                    pending = (s, b0, nb, ext, pt_sb, acc_of[s])
                if pending is not None:
                    emit_pv(*pending)

    nc.compile()
    return nc


def _host_inputs(x, Wq, bq, Wk, bk, Wv, bv):
    """Project q/k/v on the host and build the 8 per-core input maps.

    Key-chunk permutation: within each 4-chunk group g (keys [512g, 512g+512))
    parity-1 cores see chunk order [2,3,0,1]. This puts the triangular mask
    chunks of every diagonal batch at batch positions {0,1} for BOTH
    parities (positions {2,3} are then all-allowed on parity 1 and
    all-masked on parity 0), so the mask data is a single [P,2,SQ] triangle
    pair plus one per-core constant. Attention is permutation-invariant over
    keys as long as kt and v are permuted identically.

    kt layout (not duplicated): group g occupies columns [256g, 256g+256);
    rows 0-63 hold chunk positions {0,1}, rows 64-127 hold {2,3}:
      kt_data[64*(pos//2):+64, 256g + 128*(pos%2):+128] = K^T of chunk
      (g, perm[pos]).
    """
    q = x @ Wq + bq        # [B, S, H] f32
    k = x @ Wk + bk
    v = x @ Wv + bv

    r = np.arange(P)[:, None]
    f = np.arange(SQ)[None, :]
    m0 = (r <= f)
    m1 = (r + P <= f)
    tri = np.where(np.stack([m0, m1]), C2, C2 + MASK_NEG).astype(
        np.float32).transpose(1, 0, 2)          # [P, 2, SQ]
    cval = [C2 + MASK_NEG, C2]                  # parity 0, parity 1
    perm = [(0, 1, 2, 3), (2, 3, 0, 1)]

    in_maps = []
    for c in range(8):
        b, p = divmod(c, 2)
        rows = np.concatenate(
            [np.arange(u * SQ, (u + 1) * SQ) for u in range(p, 16, 2)]
        )
        qt = q[b][rows].T            # [H, NQ]
        qt = np.concatenate([qt, qt], axis=0)          # [128, NQ] dup
        ktb = k[b].T                 # [H, S]
        vb = np.concatenate([v[b], np.ones((S, 1), np.float32)], axis=1)
        qk_data = np.empty((P, NQ + S // 2), np.float32)
        vt = np.empty((P, NKC // 4, 4, H + 1), np.float32)

        def qt_col(s):
            return (s * SQ if s < 2 else
                    1024 + (s - 2) * SQ if s < 4 else
                    2048 + (s - 4) * SQ)

        def kt_col(g):
            return (512 + g * 256 if g < 2 else
                    1536 + (g - 2) * 256 if g < 4 else
                    3072 + (g - 4) * 256)

        for s in range(SLOTS):
            qk_data[:, qt_col(s):qt_col(s) + SQ] = qt[:, s * SQ:(s + 1) * SQ]
        for g in range(NKC // 4):
            for pos in range(4):
                oc = 4 * g + perm[p][pos]
                c0 = kt_col(g) + (pos % 2) * P
                qk_data[
                    (pos // 2) * H:(pos // 2 + 1) * H, c0:c0 + P
                ] = ktb[:, oc * P:(oc + 1) * P]
                vt[:, g, pos, :] = vb[oc * P:(oc + 1) * P, :]
        in_maps.append({
            "qk": np.ascontiguousarray(qk_data.astype(_BF16)),
            "vt": np.ascontiguousarray(vt.astype(_BF16)),
            "tri": tri,
            "cvec": np.full((P, 1), cval[p], np.float32),
        })
    return in_maps


def _unshard(results):
    out = np.empty((B, S, H), np.float32)
    for c in range(8):
        b, p = divmod(c, 2)
        oc = results[c]["out"]
        for si, u in enumerate(range(p, 16, 2)):
            acc = oc[si]  # [H+1, SQ]
            out[b, u * SQ:(u + 1) * SQ, :] = (acc[:H] / acc[H:H + 1]).T
    return out


def _get_runner(nc):
    """Build (once) a jitted 8-core executor for nc; returns a function
    taking in_maps and returning per-core output dicts. Mirrors
    bass2jax.run_bass_via_pjrt but caches the jit across calls."""
    import jax
    from jax.sharding import Mesh, PartitionSpec
    from jax.experimental.shard_map import shard_map
    from concourse import mybir
    from concourse.bass2jax import (
        _bass_exec_p,
        install_neuronx_cc_hook,
        partition_id_tensor,
    )

    install_neuronx_cc_hook()
    n_cores = 8
    partition_name = (
        nc.partition_id_tensor.name if nc.partition_id_tensor else None
    )
    in_names, out_names, out_avals = [], [], []
    for alloc in nc.m.functions[0].allocations:
        if not isinstance(alloc, mybir.MemoryLocationSet):
            continue
        name = alloc.memorylocations[0].name
        if alloc.kind == "ExternalInput":
            if name != partition_name:
                in_names.append(name)
        elif alloc.kind == "ExternalOutput":
            out_names.append(name)
            out_avals.append(
                jax.core.ShapedArray(
                    tuple(alloc.tensor_shape), mybir.dt.np(alloc.dtype)
                )
            )
    n_params = len(in_names)
    all_names = list(in_names) + list(out_names)
    if partition_name is not None:
        all_names.append(partition_name)

    def _body(*args):
        operands = list(args)
        if partition_name is not None:
            operands.append(partition_id_tensor())
        outs = _bass_exec_p.bind(
            *operands,
            out_avals=tuple(out_avals),
            in_names=tuple(all_names),
            out_names=tuple(out_names),
            lowering_input_output_aliases=(),
            sim_require_finite=True,
            sim_require_nnan=True,
            nc=nc,
        )
        return tuple(outs)

    devices = jax.devices()[:n_cores]
    mesh = Mesh(np.asarray(devices), ("core",))
    nouts = len(out_names)
    sharded = jax.jit(
        shard_map(
            _body,
            mesh=mesh,
            in_specs=(PartitionSpec("core"),) * (n_params + nouts),
            out_specs=(PartitionSpec("core"),) * nouts,
            check_rep=False,
        ),
        keep_unused=True,
    )

    def run(in_maps):
        concat_in = [
            np.concatenate(
                [np.asarray(in_maps[c][name]) for c in range(n_cores)], axis=0
            )
            for name in in_names
        ]
        concat_zero = [
            np.zeros((n_cores * av.shape[0], *av.shape[1:]), av.dtype)
            for av in out_avals
        ]
        outs = sharded(*concat_in, *concat_zero)
        return [
            {
                name: np.asarray(outs[i]).reshape(
                    n_cores, *out_avals[i].shape
                )[c]
                for i, name in enumerate(out_names)
            }
            for c in range(n_cores)
        ]

    return run


def kernel(x, Wq, bq, Wk, bk, Wv, bv):
    x = np.asarray(x, np.float32)
    Wq = np.asarray(Wq, np.float32)
    bq = np.asarray(bq, np.float32)
    Wk = np.asarray(Wk, np.float32)
    bk = np.asarray(bk, np.float32)
    Wv = np.asarray(Wv, np.float32)
    bv = np.asarray(bv, np.float32)

    if "nc" not in _cache:
        _cache["nc"] = _build_bass()
    nc = _cache["nc"]

    in_maps = _host_inputs(x, Wq, bq, Wk, bk, Wv, bv)
    try:
        if "runner" not in _cache:
            _cache["runner"] = _get_runner(nc)
        results = _cache["runner"](in_maps)
    except Exception:
        # fall back to the stock execution path
        _cache.pop("runner", None)
        from concourse.bass_utils import run_bass_kernel_spmd

        results = run_bass_kernel_spmd(
            nc, in_maps, core_ids=list(range(8))
        ).results
    return _unshard(results)
